# revision 11
# baseline (speedup 1.0000x reference)
"""TRN2 Bass kernel for nn_DynamicWeightProjection.

Computes, for x = query_vec reshaped [B*T, D]:
    h   = gelu_exact(x @ W1)            W1 = dw1[:, 0, {0,2}, :]   -> 256 cols
    w_c = h_c @ qkw_c                   qkw_c = qkw[0, c] reshaped [128, 128]
    out = concat(rms(w_pre[:2]), rms(w_pre[2:])*s, tanh(x@dd)[0:32],
                 rms(w_post[:2]), rms(w_post[2:])*s, tanh(x@dd)[64:96])
Only C-splits {0, 2} and dd columns {0:32, 64:96} survive into the output,
so the fused first matmul needs just 320 of the 640 columns.

Strategy: 8-way data parallel over rows (B*T = 16384 -> 2048 rows/core).
The host pre-transposes x so the contraction dim (D) lands on SBUF
partitions; everything on-chip is then transpose-free except a tiny
PE-transpose of the 64-wide dd block. The big matmul runs in float32r
(fp32 with 11-bit mantissa, full PE rate); mm2 runs in exact fp32.
The per-row-chunk postprocessing (mm2 + rms + pack) is software-pipelined
one row-chunk behind mm1 so PE's FIFO never blocks on the ACT chain.
"""
import numpy as np
from contextlib import ExitStack

import concourse.bacc as bacc
import concourse.mybir as mybir
import concourse.tile as tile
from concourse.bass_utils import run_bass_kernel_spmd

AF = mybir.ActivationFunctionType
F32 = mybir.dt.float32
F32R = mybir.dt.float32r

B, T, D = 4, 4096, 4096
NCORES = 8
ROWS = (B * T) // NCORES        # 2048 rows per core
RC = 512                        # rows per row-chunk (PSUM accumulation unit)
NRC = ROWS // RC                # 4
DC = D // 128                   # 32 contraction chunks
DCG = 4                         # d-chunks per x DMA tile
NDCG = DC // DCG                # 4
WCOLS = 320                     # 256 w-cols (c=0,2) + 32 dd_pre + 32 dd_post
EPS = 1.1920929e-07


def round_f32r(a: np.ndarray) -> np.ndarray:
    """Round fp32 -> fp32r (clear low 12 mantissa bits, round to nearest even)."""
    try:
        from neuron_dtypes import static_cast_fp32_to_fp32r, static_cast_fp32r_to_fp32
        return np.asarray(
            static_cast_fp32r_to_fp32(static_cast_fp32_to_fp32r(a)), dtype=np.float32
        ).reshape(a.shape)
    except Exception:
        u = np.ascontiguousarray(a, dtype=np.float32).view(np.uint32)
        lsb = (u >> np.uint32(12)) & np.uint32(1)
        u2 = (u + np.uint32(0x7FF) + lsb) & np.uint32(0xFFFFF000)
        return u2.view(np.float32).reshape(a.shape)


def build_nc(nrc=NRC, rc=RC, s2_scale=31250.0, s2_bias=EPS * 1e6, act=None):
    """Build the per-core SPMD program. s2_scale/s2_bias fold norm_scale into
    the w2 rms factor: rms(v)*s == 1/sqrt(ssum/(32 s^2) + eps/s^2)."""
    if act is None:
        act = AF.Gelu
    nc = bacc.Bacc("TRN2", target_bir_lowering=False, debug=False,
                   num_devices=NCORES, enable_partition_id=False)
    rows = nrc * rc
    nrbl = rc // 128

    xt_in = nc.dram_tensor("xt", [128, DC, rows], F32R, kind="ExternalInput")
    wall_in = nc.dram_tensor("wall", [128, DC, WCOLS], F32R, kind="ExternalInput")
    qkw_in = nc.dram_tensor("qkw2", [128, 2, 128], F32, kind="ExternalInput")
    id_in = nc.dram_tensor("ident", [64, 64], F32, kind="ExternalInput")
    out_d = nc.dram_tensor("out", [rows, WCOLS], F32, kind="ExternalOutput")

    with tile.TileContext(nc) as tc, ExitStack() as ctx:
        consts = ctx.enter_context(tc.tile_pool(name="consts", bufs=1))
        xpool = ctx.enter_context(tc.tile_pool(name="x", bufs=14))
        hpool = ctx.enter_context(tc.tile_pool(name="h", bufs=2))
        wpool = ctx.enter_context(tc.tile_pool(name="w", bufs=4))
        spool = ctx.enter_context(tc.tile_pool(name="s", bufs=3))
        papool = ctx.enter_context(tc.tile_pool(name="pack", bufs=2))
        ph = ctx.enter_context(tc.tile_pool(name="ph", bufs=2, space="PSUM"))
        po = ctx.enter_context(tc.tile_pool(name="po", bufs=2, space="PSUM"))

        # weight chunks: small first chunk so the very first matmuls only wait
        # on ~0.8 MiB of DMA; x tile dc-groups for the first row-chunk likewise.
        wall_sb = consts.tile([128, DC, WCOLS], F32R)
        wall_groups = [(0, 2), (2, 2)] + [(g, DCG) for g in range(DCG, DC, DCG)]
        first_groups = [(0, 2), (2, 2)] + [(g, DCG) for g in range(DCG, DC, DCG)]
        norm_groups = [(g, DCG) for g in range(0, DC, DCG)]

        nc.sync.dma_start(wall_sb[:, 0:2, :], wall_in[:, 0:2, :])

        # row-chunks: big in steady state, small at the end to shrink the tail
        if rows % 512 == 0 and rows >= 1536:
            chunks = [512] * (rows // 512 - 1) + [256, 256]
        else:
            chunks = []
            left = rows
            while left > 0:
                c = min(512, left)
                chunks.append(c)
                left -= c
        starts = [sum(chunks[:i]) for i in range(len(chunks))]

        def load_x(row0, rcl, groups):
            tiles = []
            for g0, glen in groups:
                xt = xpool.tile([128, glen, rcl], F32R, tag="xt")
                nc.sync.dma_start(
                    xt[:], xt_in[:, g0:g0 + glen, row0:row0 + rcl])
                tiles.append((g0, glen, xt))
            return tiles

        # interleave weight chunks with rc0's x tiles in dc order so the DMA
        # stream feeds PE in consumption order during the prologue
        first_tiles = []
        wrest = list(wall_groups[1:])
        for k, (xg0, xglen) in enumerate(first_groups):
            first_tiles += load_x(0, chunks[0], [(xg0, xglen)])
            if k < len(wrest):
                wg0, wglen = wrest[k]
                nc.sync.dma_start(wall_sb[:, wg0:wg0 + wglen, :],
                                  wall_in[:, wg0:wg0 + wglen, :])
        qkw_sb = consts.tile([128, 2, 128], F32)
        nc.sync.dma_start(qkw_sb[:], qkw_in[:])
        id_sb = consts.tile([64, 64], F32)
        nc.sync.dma_start(id_sb[:], id_in[:])
        bias1 = consts.tile([128, 1], F32)
        nc.vector.memset(bias1[:], EPS)
        bias2 = consts.tile([128, 1], F32)
        nc.vector.memset(bias2[:], s2_bias)

        def make_post(row0, rcl, hT0, hT1, ddT):
            """mm2 + dd transpose + rms + pack + store for one row-chunk.
            Deferred one row-chunk so PE's FIFO isn't blocked behind the
            ACT-dependent mm2 while the next mm1 could run."""
            nrbl = rcl // 128

            def post():
                pk = papool.tile([128, nrbl, WCOLS], F32, tag="pk")
                for rb in range(nrbl):
                    rbs = slice(rb * 128, (rb + 1) * 128)
                    w_ps = po.tile([128, WCOLS], F32, tag="w")
                    nc.tensor.matmul(w_ps[:, 0:128], hT0[:, rbs], qkw_sb[:, 0, :],
                                     start=True, stop=True)
                    nc.tensor.matmul(w_ps[:, 128:256], hT1[:, rbs], qkw_sb[:, 1, :],
                                     start=True, stop=True)
                    nc.tensor.transpose(w_ps[:, 256:320], ddT[:, rbs], id_sb[:])

                    # evacuate PSUM: raw w to SBUF, dd into the pack tile
                    wsb = wpool.tile([128, 256], F32, tag="wsb")
                    nc.scalar.activation(wsb[:], w_ps[:, 0:256], AF.Copy)
                    nc.scalar.activation(pk[:, rb, 128:160], w_ps[:, 256:288], AF.Copy)
                    nc.scalar.activation(pk[:, rb, 288:320], w_ps[:, 288:320], AF.Copy)

                    # sum of squares per (c, i) group of 32
                    sq = wpool.tile([128, 2, 4, 32], F32, tag="sq")
                    ss = spool.tile([128, 8], F32, tag="ss")
                    for c in range(2):
                        wv = wsb[:, c * 128:(c + 1) * 128].rearrange(
                            "p (i m) -> p i m", m=32)
                        nc.vector.tensor_mul(sq[:, c], wv, wv)
                        nc.vector.reduce_sum(ss[:, c * 4:(c + 1) * 4],
                                             sq[:, c], axis=mybir.AxisListType.X)
                    fac = spool.tile([128, 8], F32, tag="fac")
                    ssv = ss[:].rearrange("p (g i) -> p g i", i=4)
                    facv = fac[:].rearrange("p (g i) -> p g i", i=4)
                    nc.scalar.activation(facv[:, :, 0:2], ssv[:, :, 0:2], AF.Sqrt,
                                         scale=1.0 / 32.0, bias=bias1[:, 0:1])
                    nc.scalar.activation(facv[:, :, 2:4], ssv[:, :, 2:4], AF.Sqrt,
                                         scale=s2_scale, bias=bias2[:, 0:1])
                    rfac = spool.tile([128, 8], F32, tag="rfac")
                    nc.vector.reciprocal(rfac[:], fac[:])

                    for c in range(2):
                        obase = 0 if c == 0 else 160
                        for i in range(4):
                            nc.vector.tensor_scalar_mul(
                                pk[:, rb, obase + i * 32: obase + (i + 1) * 32],
                                wsb[:, c * 128 + i * 32: c * 128 + (i + 1) * 32],
                                rfac[:, c * 4 + i: c * 4 + i + 1])

                out_view = out_d[row0:row0 + rcl, :].rearrange(
                    "(rb p) c -> p rb c", p=128)
                nc.sync.dma_start(out_view, pk[:])
            return post

        pending = None
        for ci, (row0, rcl) in enumerate(zip(starts, chunks)):
            if ci == 0:
                tiles = first_tiles
            else:
                tiles = load_x(row0, rcl, norm_groups)

            h0 = ph.tile([128, rcl], F32, tag="h0")
            h1 = ph.tile([128, rcl], F32, tag="h1")
            h2 = ph.tile([64, rcl], F32, tag="h2")
            for g0, glen, xt in tiles:
                for l in range(glen):
                    dc = g0 + l
                    rhs = xt[:, l, :]
                    st, sp = dc == 0, dc == DC - 1
                    nc.tensor.matmul(h0[:], wall_sb[:, dc, 0:128], rhs, start=st, stop=sp)
                    nc.tensor.matmul(h1[:], wall_sb[:, dc, 128:256], rhs, start=st, stop=sp)
                    nc.tensor.matmul(h2[:], wall_sb[:, dc, 256:320], rhs, start=st, stop=sp)

            hT0 = hpool.tile([128, rcl], F32, tag="hT0")
            nc.scalar.activation(hT0[:], h0[:], act)
            hT1 = hpool.tile([128, rcl], F32, tag="hT1")
            nc.scalar.activation(hT1[:], h1[:], act)
            ddT = hpool.tile([64, rcl], F32, tag="ddT")
            nc.scalar.activation(ddT[:], h2[:], AF.Tanh)

            if pending is not None:
                pending()
            pending = make_post(row0, rcl, hT0, hT1, ddT)
        pending()

    nc.compile()
    return nc


def host_prep(query_vec, dw1, qkw, dd, norm_scale, nrc=NRC, rc=RC):
    """Build per-core input maps (plus shared weight arrays)."""
    rows_core = nrc * rc
    x = np.ascontiguousarray(query_vec.reshape(B * T, D), dtype=np.float32)
    x = round_f32r(x)

    w1 = dw1[:, 0, 0, :]            # [D, 128]  pre_q
    w3 = dw1[:, 0, 2, :]            # [D, 128]  post_q
    ddp = dd[:, 0, 0:32]            # [D, 32]   pre_qdd
    ddq = dd[:, 0, 64:96]           # [D, 32]   post_qdd
    w_all = np.concatenate([w1, w3, ddp, ddq], axis=1).astype(np.float32)  # [D, 320]
    w_all = round_f32r(np.ascontiguousarray(w_all))
    wall_h = np.ascontiguousarray(
        w_all.reshape(DC, 128, WCOLS).transpose(1, 0, 2))       # [128, DC, 320]

    qkw2 = np.ascontiguousarray(
        qkw[0, [0, 2]].reshape(2, 128, 128).transpose(1, 0, 2)
    ).astype(np.float32)                                         # [128, 2, 128]
    ident = np.eye(64, dtype=np.float32)

    in_maps = []
    for c in range(NCORES):
        xc = x[c * rows_core:(c + 1) * rows_core]               # [rows, D]
        xt = np.ascontiguousarray(
            xc.reshape(rows_core, DC, 128).transpose(2, 1, 0))  # [128, DC, rows]
        in_maps.append({"xt": xt, "wall": wall_h, "qkw2": qkw2, "ident": ident})
    return in_maps


_NC_CACHE = {}


def get_nc(norm_scale):
    s = float(np.asarray(norm_scale).reshape(-1)[0])
    key = (s,)
    if key not in _NC_CACHE:
        _NC_CACHE[key] = build_nc(s2_scale=1.0 / (32.0 * s * s), s2_bias=EPS / (s * s))
    return _NC_CACHE[key]


def kernel(query_vec, dw1, qkw, dd, norm_scale, _trace=False):
    nc = get_nc(norm_scale)
    in_maps = host_prep(query_vec, dw1, qkw, dd, norm_scale)
    res = run_bass_kernel_spmd(nc, in_maps, list(range(NCORES)), trace=_trace)
    out = np.concatenate([res.results[c]["out"] for c in range(NCORES)], axis=0)
    out = out.reshape(B, T, WCOLS)
    if _trace:
        kernel._last_exec_time_ns = res.exec_time_ns
        kernel._last_results = res
    return out
